# revision 7
# baseline (speedup 1.0000x reference)
"""Trainium2 Bass kernel for BalancedSpikingCell (LIF E/I network single step).

Math (see reference):
    A_cat = [x | spikes_e_prev | -spikes_i_prev]              # [B, 6144]
    W_cat = [[W_e_in | relu(W_ee) | relu(W_ei)],              # [5120, 6144]
             [W_i_in | relu(W_ie) | relu(W_ii)]]
    I_cat = A_cat @ W_cat.T                                   # [B, 5120]
    per neuron n (tau = 0.05 for e-cols, 0.1 for i-cols):
        v_dec = v + tau * (i - v)
        z     = (v_dec > 1.0)
        v_new = (1 - z) * v_dec
        i_new = 0.8 * i + I_cat

Sharding: 2-way over batch x 4-way over output neurons on 8 NeuronCores
(disjoint output blocks, no collectives). On-device layout keeps neurons on
partitions so LIF tau becomes a per-partition scalar. Matmul operands are
bf16 (spikes are exactly representable; weight rounding error ~2e-3 only
enters i_new linearly; z/v_new are computed bitwise-exactly in fp32 from v,i).
"""

import numpy as np
import ml_dtypes

BF16 = ml_dtypes.bfloat16

# Problem shapes (hardcoded per spec).
N_E, N_I, D_IN, B = 4096, 1024, 1024, 2048
NTOT = N_E + N_I          # 5120 output neurons
KTOT = D_IN + N_E + N_I   # 6144 contraction dim
R, C = 2, 4               # batch x neuron core grid (R*C == 8)
BS = B // R               # 1024 batch rows per core
NS = NTOT // C            # 1280 output neurons per core
P = 128
KT = KTOT // P            # 48 contraction tiles
MT = NS // P              # 10 neuron tiles per core
NF = 512                  # matmul moving free dim / PSUM bank
NB = BS // NF             # 2 batch chunks per core

TAU_E = float(np.float32(1.0 * (1.0 / 20.0)))   # DT * TAU_MEM_E_INV
TAU_I = float(np.float32(1.0 * (1.0 / 10.0)))   # DT * TAU_MEM_I_INV
I_DECAY = float(np.float32(1.0 - 1.0 * (1.0 / 5.0)))  # 1 - DT*TAU_SYN_INV

_CACHE = {}


def _build_nc():
    import concourse.mybir as mybir
    import concourse.tile as tile
    from concourse import bacc

    f32 = mybir.dt.float32
    bf16 = mybir.dt.bfloat16
    Alu = mybir.AluOpType

    nc = bacc.Bacc(
        "TRN2",
        target_bir_lowering=False,
        debug=False,
        enable_asserts=False,
        num_devices=8,
    )

    AT = nc.dram_tensor("at", [KTOT, BS], bf16, kind="ExternalInput").ap()
    WT = nc.dram_tensor("wt", [MT, P, KT, P], bf16, kind="ExternalInput").ap()
    V = nc.dram_tensor("v", [NS, BS], f32, kind="ExternalInput").ap()
    I = nc.dram_tensor("istate", [NS, BS], f32, kind="ExternalInput").ap()
    TAU = nc.dram_tensor("tau", [P, MT], f32, kind="ExternalInput").ap()
    Z = nc.dram_tensor("z", [NS, BS], f32, kind="ExternalOutput").ap()
    VN = nc.dram_tensor("vn", [NS, BS], f32, kind="ExternalOutput").ap()
    IN = nc.dram_tensor("inew", [NS, BS], f32, kind="ExternalOutput").ap()

    with tile.TileContext(nc) as tc:
        with (
            tc.tile_pool(name="persist", bufs=1) as persist,
            tc.tile_pool(name="wcol", bufs=2) as wpool,
            tc.tile_pool(name="state", bufs=3) as spool,
            tc.tile_pool(name="temps", bufs=4) as tpool,
            tc.tile_pool(name="outs", bufs=3) as opool,
            tc.tile_pool(name="psum", bufs=4, space="PSUM") as ppool,
        ):
            # Activations resident in SBUF: [128, KT, BS] bf16 (96 KB/partition).
            at_sb = persist.tile([P, KT, BS], bf16)
            for k in range(KT):
                nc.sync.dma_start(out=at_sb[:, k, :], in_=AT[k * P:(k + 1) * P, :])
            tau_sb = persist.tile([P, MT], f32)
            nc.sync.dma_start(out=tau_sb, in_=TAU)

            for m in range(MT):
                # Weight column for this neuron tile: [128, KT, 128] bf16.
                wcol = wpool.tile([P, KT, P], bf16, tag="wcol")
                nc.sync.dma_start(out=wcol, in_=WT[m])

                ps = []
                for n in range(NB):
                    pt = ppool.tile([P, NF], f32, tag=f"ps{n}", name=f"ps_{m}_{n}")
                    ps.append(pt)
                for k in range(KT):
                    for n in range(NB):
                        nc.tensor.matmul(
                            ps[n],
                            wcol[:, k, :],
                            at_sb[:, k, n * NF:(n + 1) * NF],
                            start=(k == 0),
                            stop=(k == KT - 1),
                        )

                tau_ap = tau_sb[:, m:m + 1]
                for n in range(NB):
                    rs = slice(m * P, (m + 1) * P)
                    cs = slice(n * NF, (n + 1) * NF)
                    v_sb = spool.tile([P, NF], f32, tag="v", name=f"v_{m}_{n}")
                    i_sb = spool.tile([P, NF], f32, tag="i", name=f"i_{m}_{n}")
                    nc.sync.dma_start(out=v_sb, in_=V[rs, cs])
                    nc.sync.dma_start(out=i_sb, in_=I[rs, cs])

                    t1 = tpool.tile([P, NF], f32, tag="t1", name=f"t1_{m}_{n}")
                    vd = tpool.tile([P, NF], f32, tag="vd", name=f"vd_{m}_{n}")
                    # t1 = i - v ; vd = tau*t1 + v   (bitwise == reference order)
                    nc.vector.tensor_sub(t1, i_sb, v_sb)
                    nc.vector.scalar_tensor_tensor(vd, t1, tau_ap, v_sb, Alu.mult, Alu.add)

                    z_sb = opool.tile([P, NF], f32, tag="z", name=f"z_{m}_{n}")
                    vn_sb = opool.tile([P, NF], f32, tag="vn", name=f"vn_{m}_{n}")
                    in_sb = opool.tile([P, NF], f32, tag="in", name=f"in_{m}_{n}")
                    # z = (vd > 1);  v_new = (vd <= 1) * vd;  i_new = 0.8*i + I
                    nc.vector.tensor_scalar(z_sb, vd, 1.0, None, Alu.is_gt)
                    nc.vector.scalar_tensor_tensor(vn_sb, vd, 1.0, vd, Alu.is_le, Alu.mult)
                    nc.vector.scalar_tensor_tensor(in_sb, i_sb, I_DECAY, ps[n], Alu.mult, Alu.add)

                    nc.sync.dma_start(out=Z[rs, cs], in_=z_sb)
                    nc.sync.dma_start(out=VN[rs, cs], in_=vn_sb)
                    nc.sync.dma_start(out=IN[rs, cs], in_=in_sb)

    nc.compile()
    return nc


def _prep_inputs(x, v_e, i_e, v_i, i_i, spikes_e_prev, spikes_i_prev,
                 W_ee, W_ie, W_ei, W_ii, W_e_in, W_i_in):
    f32 = np.float32

    # Activations, transposed to [K, B], bf16.
    at = np.empty((KTOT, B), BF16)
    at[:D_IN] = x.T
    at[D_IN:D_IN + N_E] = spikes_e_prev.T
    at[D_IN + N_E:] = -spikes_i_prev.T

    # Combined transposed weights [K, NTOT], bf16 (relu applied to recurrents).
    wt = np.empty((KTOT, NTOT), BF16)
    wt[:D_IN, :N_E] = W_e_in.T
    wt[:D_IN, N_E:] = W_i_in.T
    wt[D_IN:D_IN + N_E, :N_E] = np.maximum(W_ee, 0.0).T
    wt[D_IN:D_IN + N_E, N_E:] = np.maximum(W_ie, 0.0).T
    wt[D_IN + N_E:, :N_E] = np.maximum(W_ei, 0.0).T
    wt[D_IN + N_E:, N_E:] = np.maximum(W_ii, 0.0).T

    v_cat = np.concatenate([v_e, v_i], axis=1).astype(f32, copy=False)
    i_cat = np.concatenate([i_e, i_i], axis=1).astype(f32, copy=False)

    tau_full = np.empty(NTOT, f32)
    tau_full[:N_E] = np.float32(TAU_E)
    tau_full[N_E:] = np.float32(TAU_I)

    in_maps = []
    for core in range(8):
        r, c = divmod(core, C)
        bsl = slice(r * BS, (r + 1) * BS)
        nsl = slice(c * NS, (c + 1) * NS)
        wt_c = wt[:, nsl]  # [KTOT, NS]
        # [MT, 128(kp), KT, 128(mp)] so each per-m DMA source is contiguous.
        wt_p = np.ascontiguousarray(
            wt_c.reshape(KT, P, MT, P).transpose(2, 1, 0, 3))
        in_maps.append({
            "at": np.ascontiguousarray(at[:, bsl]),
            "wt": wt_p,
            "v": np.ascontiguousarray(v_cat[bsl, nsl].T),
            "istate": np.ascontiguousarray(i_cat[bsl, nsl].T),
            "tau": np.ascontiguousarray(tau_full[nsl].reshape(MT, P).T),
        })
    return in_maps


def _install_neff_disk_cache():
    """Cache compiled NEFFs on disk keyed by BIR hash (compile is ~4 min)."""
    import hashlib
    import os
    import shutil

    from concourse import bass2jax

    if getattr(bass2jax.compile_bir_kernel, "_neff_cache_wrapped", False):
        return
    cache_dir = os.environ.get("BASS_NEFF_CACHE_DIR", "/tmp/bass_neff_cache")
    orig = bass2jax.compile_bir_kernel

    def cached(bir_json, tmpdir, neff_name="file.neff"):
        key = hashlib.sha256(
            bir_json if isinstance(bir_json, bytes) else bir_json.encode()
        ).hexdigest()
        cpath = os.path.join(cache_dir, f"{key}.neff")
        dst = os.path.join(tmpdir, "sg00", neff_name)
        if os.path.exists(cpath):
            os.makedirs(os.path.dirname(dst), exist_ok=True)
            shutil.copy(cpath, dst)
            return dst
        out = orig(bir_json, tmpdir, neff_name)
        try:
            os.makedirs(cache_dir, exist_ok=True)
            tmp = cpath + ".tmp"
            shutil.copy(out, tmp)
            os.replace(tmp, cpath)
        except OSError:
            pass
        return out

    cached._neff_cache_wrapped = True
    bass2jax.compile_bir_kernel = cached


class _Runner:
    """Persistent jitted SPMD executor (mirrors bass2jax.run_bass_via_pjrt,
    but reuses one jitted callable across calls and skips output donation —
    this kernel writes every output element)."""

    def __init__(self, nc, n_cores=8):
        import concourse.mybir as mybir
        import jax
        import numpy as np
        from concourse import bass2jax
        from jax.sharding import Mesh, PartitionSpec
        from jax.experimental.shard_map import shard_map

        _install_neff_disk_cache()
        bass2jax.install_neuronx_cc_hook()

        self.n_cores = n_cores
        partition_name = (
            nc.partition_id_tensor.name if nc.partition_id_tensor else None)
        in_names, out_names, out_avals = [], [], []
        for alloc in nc.m.functions[0].allocations:
            if not isinstance(alloc, mybir.MemoryLocationSet):
                continue
            name = alloc.memorylocations[0].name
            if alloc.kind == "ExternalInput":
                if name != partition_name:
                    in_names.append(name)
            elif alloc.kind == "ExternalOutput":
                out_names.append(name)
                out_avals.append(jax.core.ShapedArray(
                    tuple(alloc.tensor_shape), mybir.dt.np(alloc.dtype)))
        self.in_names, self.out_names, self.out_avals = in_names, out_names, out_avals
        n_params = len(in_names)
        all_in_names = in_names + out_names
        if partition_name is not None:
            all_in_names = all_in_names + [partition_name]
        all_in_names = tuple(all_in_names)
        self.zeros = [np.zeros((n_cores * a.shape[0], *a.shape[1:]), a.dtype)
                      for a in out_avals]

        def _body(*args):
            operands = list(args)
            if partition_name is not None:
                operands.append(bass2jax.partition_id_tensor())
            outs = bass2jax._bass_exec_p.bind(
                *operands,
                out_avals=tuple(out_avals),
                in_names=all_in_names,
                out_names=tuple(out_names),
                lowering_input_output_aliases=(),
                sim_require_finite=True,
                sim_require_nnan=True,
                nc=nc,
            )
            return tuple(outs)

        devices = jax.devices()[:n_cores]
        self.mesh = Mesh(np.asarray(devices), ("core",))
        self.pspec = PartitionSpec("core")
        n_all = n_params + len(out_names)
        self.sharded = jax.jit(
            shard_map(
                _body, mesh=self.mesh,
                in_specs=(self.pspec,) * n_all,
                out_specs=(self.pspec,) * len(out_names),
                check_rep=False,
            ),
            keep_unused=True,
        )

    def concat_inputs(self, in_maps):
        import numpy as np
        return [np.concatenate([m[name] for m in in_maps], axis=0)
                for name in self.in_names]

    def execute(self, concat_in):
        return self.sharded(*concat_in, *self.zeros)

    def run(self, in_maps):
        import numpy as np
        out_arrs = self.execute(self.concat_inputs(in_maps))
        return [
            {name: np.asarray(out_arrs[i]).reshape(
                self.n_cores, *self.out_avals[i].shape)[c]
             for i, name in enumerate(self.out_names)}
            for c in range(self.n_cores)
        ]


def _get_runner():
    if "runner" not in _CACHE:
        _CACHE["runner"] = _Runner(_build_nc())
    return _CACHE["runner"]


def _run(in_maps):
    return _get_runner().run(in_maps)


def kernel(**inputs):
    inputs = {k: np.asarray(v) for k, v in inputs.items()}
    in_maps = _prep_inputs(**inputs)
    res = _run(in_maps)

    out_z = np.empty((B, NTOT), np.float32)
    out_vn = np.empty((B, NTOT), np.float32)
    out_in = np.empty((B, NTOT), np.float32)
    for core in range(8):
        r, c = divmod(core, C)
        bsl = slice(r * BS, (r + 1) * BS)
        nsl = slice(c * NS, (c + 1) * NS)
        out_z[bsl, nsl] = res[core]["z"].T
        out_vn[bsl, nsl] = res[core]["vn"].T
        out_in[bsl, nsl] = res[core]["inew"].T

    spikes_e = np.ascontiguousarray(out_z[:, :N_E])
    spikes_i = np.ascontiguousarray(out_z[:, N_E:])
    v_e_new = np.ascontiguousarray(out_vn[:, :N_E])
    i_e_new = np.ascontiguousarray(out_in[:, :N_E])
    v_i_new = np.ascontiguousarray(out_vn[:, N_E:])
    i_i_new = np.ascontiguousarray(out_in[:, N_E:])
    return spikes_e, spikes_i, v_e_new, i_e_new, v_i_new, i_i_new


# revision 10
# speedup vs baseline: 188.1543x; 188.1543x over previous
"""Trainium2 Bass kernel for BalancedSpikingCell (LIF E/I network single step).

Math (see reference):
    A_cat = [x | spikes_e_prev | -spikes_i_prev]              # [B, 6144]
    W_cat = [[W_e_in | relu(W_ee) | relu(W_ei)],              # [5120, 6144]
             [W_i_in | relu(W_ie) | relu(W_ii)]]
    I_cat = A_cat @ W_cat.T                                   # [B, 5120]
    per neuron n (tau = 0.05 for e-cols, 0.1 for i-cols):
        v_dec = v + tau * (i - v)
        z     = (v_dec > 1.0)
        v_new = (1 - z) * v_dec
        i_new = 0.8 * i + I_cat

Sharding: 2-way over batch x 4-way over output neurons on 8 NeuronCores
(disjoint output blocks, no collectives). On-device layout keeps neurons on
partitions so LIF tau becomes a per-partition scalar. Matmul operands are
bf16 (spikes are exactly representable; weight rounding error ~2e-3 only
enters i_new linearly; z/v_new are computed bitwise-exactly in fp32 from v,i).
"""

import numpy as np
import ml_dtypes

BF16 = ml_dtypes.bfloat16

# Problem shapes (hardcoded per spec).
N_E, N_I, D_IN, B = 4096, 1024, 1024, 2048
NTOT = N_E + N_I          # 5120 output neurons
KTOT = D_IN + N_E + N_I   # 6144 contraction dim
R, C = 2, 4               # batch x neuron core grid (R*C == 8)
BS = B // R               # 1024 batch rows per core
NS = NTOT // C            # 1280 output neurons per core
P = 128
KT = KTOT // P            # 48 contraction tiles
MT = NS // P              # 10 neuron tiles per core
NF = 512                  # matmul moving free dim / PSUM bank
NB = BS // NF             # 2 batch chunks per core

TAU_E = float(np.float32(1.0 * (1.0 / 20.0)))   # DT * TAU_MEM_E_INV
TAU_I = float(np.float32(1.0 * (1.0 / 10.0)))   # DT * TAU_MEM_I_INV
I_DECAY = float(np.float32(1.0 - 1.0 * (1.0 / 5.0)))  # 1 - DT*TAU_SYN_INV

_CACHE = {}


def _build_nc(reps=1):
    import concourse.mybir as mybir
    import concourse.tile as tile
    from concourse import bacc

    f32 = mybir.dt.float32
    bf16 = mybir.dt.bfloat16
    Alu = mybir.AluOpType

    nc = bacc.Bacc(
        "TRN2",
        target_bir_lowering=False,
        debug=False,
        enable_asserts=False,
        num_devices=8,
    )

    AT = nc.dram_tensor("at", [KTOT, BS], bf16, kind="ExternalInput").ap()
    WT = nc.dram_tensor("wt", [MT, P, KT, P], bf16, kind="ExternalInput").ap()
    V = nc.dram_tensor("v", [NS, BS], f32, kind="ExternalInput").ap()
    I = nc.dram_tensor("istate", [NS, BS], f32, kind="ExternalInput").ap()
    TAU = nc.dram_tensor("tau", [P, MT], f32, kind="ExternalInput").ap()
    Z = nc.dram_tensor("z", [NS, BS], f32, kind="ExternalOutput").ap()
    VN = nc.dram_tensor("vn", [NS, BS], f32, kind="ExternalOutput").ap()
    IN = nc.dram_tensor("inew", [NS, BS], f32, kind="ExternalOutput").ap()

    with tile.TileContext(nc) as tc:
        with (
            tc.tile_pool(name="persist", bufs=1) as persist,
            tc.tile_pool(name="wcol", bufs=2) as wpool,
            tc.tile_pool(name="state", bufs=3) as spool,
            tc.tile_pool(name="temps", bufs=4) as tpool,
            tc.tile_pool(name="outs", bufs=3) as opool,
            tc.tile_pool(name="psum", bufs=4, space="PSUM") as ppool,
        ):
            def body():
                # Activations resident in SBUF: [128, KT, BS] bf16 (96 KB/p).
                at_sb = persist.tile([P, KT, BS], bf16, name="at_sb")
                for k in range(KT):
                    nc.sync.dma_start(out=at_sb[:, k, :], in_=AT[k * P:(k + 1) * P, :])
                tau_sb = persist.tile([P, MT], f32, name="tau_sb")
                nc.sync.dma_start(out=tau_sb, in_=TAU)

                for m in range(MT):
                    # Weight column for this neuron tile: [128, KT, 128] bf16.
                    wcol = wpool.tile([P, KT, P], bf16, tag="wcol", name=f"wcol_{m}")
                    nc.sync.dma_start(out=wcol, in_=WT[m])

                    ps = []
                    for n in range(NB):
                        pt = ppool.tile([P, NF], f32, tag=f"ps{n}", name=f"ps_{m}_{n}")
                        ps.append(pt)
                    for k in range(KT):
                        for n in range(NB):
                            nc.tensor.matmul(
                                ps[n],
                                wcol[:, k, :],
                                at_sb[:, k, n * NF:(n + 1) * NF],
                                start=(k == 0),
                                stop=(k == KT - 1),
                            )

                    tau_ap = tau_sb[:, m:m + 1]
                    for n in range(NB):
                        rs = slice(m * P, (m + 1) * P)
                        cs = slice(n * NF, (n + 1) * NF)
                        v_sb = spool.tile([P, NF], f32, tag="v", name=f"v_{m}_{n}")
                        i_sb = spool.tile([P, NF], f32, tag="i", name=f"i_{m}_{n}")
                        nc.sync.dma_start(out=v_sb, in_=V[rs, cs])
                        nc.sync.dma_start(out=i_sb, in_=I[rs, cs])

                        t1 = tpool.tile([P, NF], f32, tag="t1", name=f"t1_{m}_{n}")
                        vd = tpool.tile([P, NF], f32, tag="vd", name=f"vd_{m}_{n}")
                        # t1 = i - v ; vd = tau*t1 + v  (bitwise == reference)
                        nc.vector.tensor_sub(t1, i_sb, v_sb)
                        nc.vector.scalar_tensor_tensor(vd, t1, tau_ap, v_sb, Alu.mult, Alu.add)

                        z_sb = opool.tile([P, NF], f32, tag="z", name=f"z_{m}_{n}")
                        vn_sb = opool.tile([P, NF], f32, tag="vn", name=f"vn_{m}_{n}")
                        in_sb = opool.tile([P, NF], f32, tag="in", name=f"in_{m}_{n}")
                        # z = (vd > 1); v_new = (vd <= 1)*vd; i_new = 0.8*i + I
                        nc.vector.tensor_scalar(z_sb, vd, 1.0, None, Alu.is_gt)
                        nc.vector.scalar_tensor_tensor(vn_sb, vd, 1.0, vd, Alu.is_le, Alu.mult)
                        nc.vector.scalar_tensor_tensor(in_sb, i_sb, I_DECAY, ps[n], Alu.mult, Alu.add)

                        nc.sync.dma_start(out=Z[rs, cs], in_=z_sb)
                        nc.sync.dma_start(out=VN[rs, cs], in_=vn_sb)
                        nc.sync.dma_start(out=IN[rs, cs], in_=in_sb)

            if reps == 1:
                body()
            else:
                with tc.For_i(0, reps, 1):
                    body()

    nc.compile()
    return nc


def _prep_inputs(x, v_e, i_e, v_i, i_i, spikes_e_prev, spikes_i_prev,
                 W_ee, W_ie, W_ei, W_ii, W_e_in, W_i_in):
    f32 = np.float32

    # Activations, transposed to [K, B], bf16.
    at = np.empty((KTOT, B), BF16)
    at[:D_IN] = x.T
    at[D_IN:D_IN + N_E] = spikes_e_prev.T
    at[D_IN + N_E:] = -spikes_i_prev.T

    # Combined transposed weights [K, NTOT], bf16 (relu applied to recurrents).
    wt = np.empty((KTOT, NTOT), BF16)
    wt[:D_IN, :N_E] = W_e_in.T
    wt[:D_IN, N_E:] = W_i_in.T
    wt[D_IN:D_IN + N_E, :N_E] = np.maximum(W_ee, 0.0).T
    wt[D_IN:D_IN + N_E, N_E:] = np.maximum(W_ie, 0.0).T
    wt[D_IN + N_E:, :N_E] = np.maximum(W_ei, 0.0).T
    wt[D_IN + N_E:, N_E:] = np.maximum(W_ii, 0.0).T

    v_cat = np.concatenate([v_e, v_i], axis=1).astype(f32, copy=False)
    i_cat = np.concatenate([i_e, i_i], axis=1).astype(f32, copy=False)

    tau_full = np.empty(NTOT, f32)
    tau_full[:N_E] = np.float32(TAU_E)
    tau_full[N_E:] = np.float32(TAU_I)

    in_maps = []
    for core in range(8):
        r, c = divmod(core, C)
        bsl = slice(r * BS, (r + 1) * BS)
        nsl = slice(c * NS, (c + 1) * NS)
        wt_c = wt[:, nsl]  # [KTOT, NS]
        # [MT, 128(kp), KT, 128(mp)] so each per-m DMA source is contiguous.
        wt_p = np.ascontiguousarray(
            wt_c.reshape(KT, P, MT, P).transpose(2, 1, 0, 3))
        in_maps.append({
            "at": np.ascontiguousarray(at[:, bsl]),
            "wt": wt_p,
            "v": np.ascontiguousarray(v_cat[bsl, nsl].T),
            "istate": np.ascontiguousarray(i_cat[bsl, nsl].T),
            "tau": np.ascontiguousarray(tau_full[nsl].reshape(MT, P).T),
        })
    return in_maps


def _install_neff_disk_cache():
    """Cache compiled NEFFs on disk keyed by BIR hash (compile is ~4 min)."""
    import hashlib
    import os
    import shutil

    from concourse import bass2jax

    if getattr(bass2jax.compile_bir_kernel, "_neff_cache_wrapped", False):
        return
    cache_dir = os.environ.get("BASS_NEFF_CACHE_DIR", "/tmp/bass_neff_cache")
    orig = bass2jax.compile_bir_kernel

    def cached(bir_json, tmpdir, neff_name="file.neff"):
        key = hashlib.sha256(
            bir_json if isinstance(bir_json, bytes) else bir_json.encode()
        ).hexdigest()
        cpath = os.path.join(cache_dir, f"{key}.neff")
        dst = os.path.join(tmpdir, "sg00", neff_name)
        if os.path.exists(cpath):
            os.makedirs(os.path.dirname(dst), exist_ok=True)
            shutil.copy(cpath, dst)
            return dst
        out = orig(bir_json, tmpdir, neff_name)
        try:
            os.makedirs(cache_dir, exist_ok=True)
            tmp = cpath + ".tmp"
            shutil.copy(out, tmp)
            os.replace(tmp, cpath)
        except OSError:
            pass
        return out

    cached._neff_cache_wrapped = True
    bass2jax.compile_bir_kernel = cached


class _Runner:
    """Persistent jitted SPMD executor (mirrors bass2jax.run_bass_via_pjrt,
    but reuses one jitted callable across calls and skips output donation —
    this kernel writes every output element)."""

    def __init__(self, nc, n_cores=8):
        import concourse.mybir as mybir
        import jax
        import numpy as np
        from concourse import bass2jax
        from jax.sharding import Mesh, PartitionSpec
        from jax.experimental.shard_map import shard_map

        _install_neff_disk_cache()
        bass2jax.install_neuronx_cc_hook()

        self.n_cores = n_cores
        partition_name = (
            nc.partition_id_tensor.name if nc.partition_id_tensor else None)
        in_names, out_names, out_avals = [], [], []
        for alloc in nc.m.functions[0].allocations:
            if not isinstance(alloc, mybir.MemoryLocationSet):
                continue
            name = alloc.memorylocations[0].name
            if alloc.kind == "ExternalInput":
                if name != partition_name:
                    in_names.append(name)
            elif alloc.kind == "ExternalOutput":
                out_names.append(name)
                out_avals.append(jax.core.ShapedArray(
                    tuple(alloc.tensor_shape), mybir.dt.np(alloc.dtype)))
        self.in_names, self.out_names, self.out_avals = in_names, out_names, out_avals
        n_params = len(in_names)
        all_in_names = in_names + out_names
        if partition_name is not None:
            all_in_names = all_in_names + [partition_name]
        all_in_names = tuple(all_in_names)
        self.zeros = [np.zeros((n_cores * a.shape[0], *a.shape[1:]), a.dtype)
                      for a in out_avals]

        def _body(*args):
            operands = list(args)
            if partition_name is not None:
                operands.append(bass2jax.partition_id_tensor())
            outs = bass2jax._bass_exec_p.bind(
                *operands,
                out_avals=tuple(out_avals),
                in_names=all_in_names,
                out_names=tuple(out_names),
                lowering_input_output_aliases=(),
                sim_require_finite=True,
                sim_require_nnan=True,
                nc=nc,
            )
            return tuple(outs)

        devices = jax.devices()[:n_cores]
        self.mesh = Mesh(np.asarray(devices), ("core",))
        self.pspec = PartitionSpec("core")
        n_all = n_params + len(out_names)
        self.sharded = jax.jit(
            shard_map(
                _body, mesh=self.mesh,
                in_specs=(self.pspec,) * n_all,
                out_specs=(self.pspec,) * len(out_names),
                check_rep=False,
            ),
            keep_unused=True,
        )

    def concat_inputs(self, in_maps):
        import numpy as np
        return [np.concatenate([m[name] for m in in_maps], axis=0)
                for name in self.in_names]

    def execute(self, concat_in):
        return self.sharded(*concat_in, *self.zeros)

    def run(self, in_maps):
        import numpy as np
        out_arrs = self.execute(self.concat_inputs(in_maps))
        return [
            {name: np.asarray(out_arrs[i]).reshape(
                self.n_cores, *self.out_avals[i].shape)[c]
             for i, name in enumerate(self.out_names)}
            for c in range(self.n_cores)
        ]


def _get_runner():
    if "runner" not in _CACHE:
        _CACHE["runner"] = _Runner(_build_nc())
    return _CACHE["runner"]


def _run(in_maps):
    return _get_runner().run(in_maps)


def kernel(**inputs):
    inputs = {k: np.asarray(v) for k, v in inputs.items()}
    in_maps = _prep_inputs(**inputs)
    res = _run(in_maps)

    out_z = np.empty((B, NTOT), np.float32)
    out_vn = np.empty((B, NTOT), np.float32)
    out_in = np.empty((B, NTOT), np.float32)
    for core in range(8):
        r, c = divmod(core, C)
        bsl = slice(r * BS, (r + 1) * BS)
        nsl = slice(c * NS, (c + 1) * NS)
        out_z[bsl, nsl] = res[core]["z"].T
        out_vn[bsl, nsl] = res[core]["vn"].T
        out_in[bsl, nsl] = res[core]["inew"].T

    spikes_e = np.ascontiguousarray(out_z[:, :N_E])
    spikes_i = np.ascontiguousarray(out_z[:, N_E:])
    v_e_new = np.ascontiguousarray(out_vn[:, :N_E])
    i_e_new = np.ascontiguousarray(out_in[:, :N_E])
    v_i_new = np.ascontiguousarray(out_vn[:, N_E:])
    i_i_new = np.ascontiguousarray(out_in[:, N_E:])
    return spikes_e, spikes_i, v_e_new, i_e_new, v_i_new, i_i_new
